# revision 3
# baseline (speedup 1.0000x reference)
"""CapsNet dynamic-routing kernel for 8 TRN2 NeuronCores.

Problem: x [256,1152,8], W [1152,10,8,16], 3 routing iterations, out [256,10,16,1].

Strategy (sharded over the input-capsule axis I, 144 capsules / core):
  u_hat is NEVER materialized (it would be 189MB). Instead each routing
  iteration computes, per core (ID = 144*8 = 1152 local (i,d) rows):
    s_partial[b,(o,e)] = x_flat[b,:] @ (c*W)_flat[:,(o,e)]     (dense matmul, K=ID)
    s = AllReduce(s_partial) over the 8 I-shards                (164KB, 1/iter)
    v = squash(s)            (computed redundantly on all cores)
    G[(i,d),(o,e)] = x_flat^T @ v_flat                          (dense matmul)
    agree[i,o] = (1/B) * sum_{d,e} (W_flat * G)[(i,d),(o,e)]    (local, no comm)
    b += agree ; c = softmax(b, axis=o)                         (local)
  Iteration 1 uses uniform c=1/O folded into the squash scale, so the
  c*W build is skipped there.  The last iteration skips the agreement.
"""

import numpy as np

B, I, O, DIN, DOUT = 256, 1152, 10, 8, 16
NCORES = 8
I_SH = I // NCORES          # 144 input capsules per core
ID = I_SH * DIN             # 1152 local (i,d) rows
NT = ID // 128              # 9 partition tiles of (i,d)
BT = B // 128               # 2 partition tiles of batch
OE = O * DOUT               # 160
ROUTING_ITERS = 3

_CACHE = {}


def _bcast_inner(ap_mod, ap, n):
    """View an AP with an extra innermost broadcast axis of length n."""
    return ap_mod.AP(tensor=ap.tensor, offset=ap.offset, ap=[*ap.ap, [0, n]])


def _build():
    import concourse.bass as bass
    import concourse.bacc as bacc
    import concourse.tile as tile
    from concourse import mybir

    f32 = mybir.dt.float32
    AF = mybir.ActivationFunctionType
    ALU = mybir.AluOpType

    nc = bacc.Bacc("TRN2", target_bir_lowering=False, debug=False,
                   num_devices=NCORES)

    # All inputs pre-tiled on host to [128, ...] so every DMA is contiguous.
    xT_d = nc.dram_tensor("xT", [128, NT, B], f32, kind="ExternalInput")
    xf_d = nc.dram_tensor("xf", [128, BT, ID], f32, kind="ExternalInput")
    W_d = nc.dram_tensor("Wf", [128, NT, OE], f32, kind="ExternalInput")
    MB_d = nc.dram_tensor("Mblk", [128, 128], f32, kind="ExternalInput")
    out_d = nc.dram_tensor("out", [B, OE], f32, kind="ExternalOutput")

    with tile.TileContext(nc) as tc:
        with (
            tc.tile_pool(name="sb", bufs=1) as sb,
            tc.tile_pool(name="work", bufs=2) as work,
            tc.tile_pool(name="ps_s", bufs=2, space="PSUM") as ps_s,
            tc.tile_pool(name="ps_g", bufs=2, space="PSUM") as ps_g,
            tc.tile_pool(name="ps_a", bufs=2, space="PSUM") as ps_a,
            tc.tile_pool(name="dram", bufs=3, space="DRAM") as dram,
        ):
            # ---- persistent SBUF tensors ----
            xT = sb.tile([128, NT, B], f32)       # x_flat^T tiles (lhsT for s)
            xf = sb.tile([128, BT, ID], f32)      # x_flat tiles (lhsT for G)
            Wf = sb.tile([128, NT, OE], f32)      # W_flat tiles
            Mblk = sb.tile([128, 128], f32)       # 8x8 block-diag ones
            bq = sb.tile([128, NT, O], f32)       # routing logits b (expanded)
            csb = sb.tile([128, NT, O], f32)      # softmax(b)
            Wc = sb.tile([128, NT, OE], f32)      # c * W
            s_sb = sb.tile([128, BT, OE], f32)    # local partial s
            sf = sb.tile([128, BT, OE], f32)      # all-reduced s
            v_sb = sb.tile([128, BT, OE], f32)    # squash(s)
            WG = sb.tile([128, NT, OE], f32)      # W * G
            A1 = sb.tile([128, NT, O], f32)       # e-reduced agreement

            nc.sync.dma_start(out=xT[:], in_=xT_d[:])
            nc.sync.dma_start(out=xf[:], in_=xf_d[:])
            nc.sync.dma_start(out=Wf[:], in_=W_d[:])
            nc.sync.dma_start(out=Mblk[:], in_=MB_d[:])

            Wf4 = Wf.rearrange("p t (o e) -> p t o e", o=O)
            Wc4 = Wc.rearrange("p t (o e) -> p t o e", o=O)
            WG4 = WG.rearrange("p t (o e) -> p t o e", o=O)
            s4 = s_sb.rearrange("p b (o e) -> p b o e", o=O)
            sf4 = sf.rearrange("p b (o e) -> p b o e", o=O)
            v4 = v_sb.rearrange("p b (o e) -> p b o e", o=O)

            for it in range(ROUTING_ITERS):
                first, last = it == 0, it == ROUTING_ITERS - 1

                if first:
                    rhs = Wf      # uniform c = 1/O folded into squash scale
                else:
                    # c = softmax(b) over o, per (i,d) row segment
                    m = work.tile([128, NT], f32, tag="m")
                    nc.vector.reduce_max(out=m[:], in_=bq[:],
                                         axis=mybir.AxisListType.X)
                    nm = work.tile([128, NT], f32, tag="nm")
                    nc.vector.tensor_scalar_mul(nm[:], m[:], -1.0)
                    ex = work.tile([128, NT, O], f32, tag="ex")
                    nc.vector.tensor_tensor(
                        out=ex[:], in0=bq[:],
                        in1=_bcast_inner(bass, nm[:], O), op=ALU.add)
                    nc.scalar.activation(out=ex[:], in_=ex[:], func=AF.Exp)
                    sm = work.tile([128, NT], f32, tag="sm")
                    nc.vector.reduce_sum(out=sm[:], in_=ex[:],
                                         axis=mybir.AxisListType.X)
                    nc.vector.reciprocal(out=sm[:], in_=sm[:])
                    nc.vector.tensor_tensor(
                        out=csb[:], in0=ex[:],
                        in1=_bcast_inner(bass, sm[:], O), op=ALU.mult)
                    # Wc = W * c  (c broadcast over e)
                    nc.vector.tensor_tensor(
                        out=Wc4[:], in0=Wf4[:],
                        in1=_bcast_inner(bass, csb[:], DOUT), op=ALU.mult)
                    rhs = Wc

                # s_partial = x_flat @ rhs : out [b-tile 128, OE]
                for bt in range(BT):
                    s_ps = ps_s.tile([128, OE], f32, tag="s_ps")
                    for k in range(NT):
                        nc.tensor.matmul(
                            s_ps[:],
                            xT[:, k, bt * 128:(bt + 1) * 128],
                            rhs[:, k, :],
                            start=(k == 0), stop=(k == NT - 1))
                    nc.vector.tensor_copy(s_sb[:, bt, :], s_ps[:])

                # AllReduce s over the 8 I-shards
                cc_in = dram.tile([BT, 128, OE], f32, tag="cc_in")
                cc_out = dram.tile([BT, 128, OE], f32, tag="cc_out",
                                   addr_space="Shared")
                nc.sync.dma_start(
                    out=cc_in.rearrange("b p f -> p b f"), in_=s_sb[:])
                nc.gpsimd.collective_compute(
                    "AllReduce", ALU.add,
                    replica_groups=[list(range(NCORES))],
                    ins=[cc_in.opt()], outs=[cc_out.opt()])
                nc.sync.dma_start(
                    out=sf[:], in_=cc_out.rearrange("b p f -> p b f"))

                # squash: v = s * sqrt(ss)/(1+ss) per (b, o); iteration 1
                # carries c=1/O as s_raw = O*s_true.
                sq = work.tile([128, BT, OE], f32, tag="sq")
                nc.scalar.activation(out=sq[:], in_=sf[:], func=AF.Square)
                ss = work.tile([128, BT, O], f32, tag="ss")
                nc.vector.reduce_sum(
                    out=ss[:], in_=sq.rearrange("p b (o e) -> p b o e", o=O),
                    axis=mybir.AxisListType.X)
                t1 = work.tile([128, BT, O], f32, tag="t1")
                nc.scalar.activation(out=t1[:], in_=ss[:], func=AF.Sqrt)
                den = work.tile([128, BT, O], f32, tag="den")
                if first:
                    # ss_raw = O^2*ss_true: v = s_raw*(1/O^2)*sqrt(ss_raw)/(1+ss_raw/O^2)
                    nc.scalar.activation(out=den[:], in_=ss[:], func=AF.Copy,
                                         scale=1.0 / (O * O), bias=1.0)
                else:
                    nc.scalar.activation(out=den[:], in_=ss[:], func=AF.Copy,
                                         scale=1.0, bias=1.0)
                nc.vector.reciprocal(out=den[:], in_=den[:])
                rat = work.tile([128, BT, O], f32, tag="rat")
                nc.vector.tensor_tensor(out=rat[:], in0=t1[:], in1=den[:],
                                        op=ALU.mult)
                if first:
                    nc.vector.tensor_scalar_mul(rat[:], rat[:], 1.0 / (O * O))
                nc.vector.tensor_tensor(
                    out=v4[:], in0=sf4[:],
                    in1=_bcast_inner(bass, rat[:], DOUT), op=ALU.mult)

                if last:
                    nc.sync.dma_start(
                        out=out_d.rearrange("(b p) f -> p b f", p=128),
                        in_=v_sb[:])
                else:
                    # G = x_flat^T @ v ; agree = (1/B) sum_de W*G ; b += agree
                    for mt in range(NT):
                        g_ps = ps_g.tile([128, OE], f32, tag="g_ps")
                        for bt in range(BT):
                            nc.tensor.matmul(
                                g_ps[:],
                                xf[:, bt, mt * 128:(mt + 1) * 128],
                                v_sb[:, bt, :],
                                start=(bt == 0), stop=(bt == BT - 1))
                        nc.vector.tensor_tensor(
                            out=WG[:, mt, :], in0=Wf[:, mt, :], in1=g_ps[:],
                            op=ALU.mult)
                    nc.vector.reduce_sum(out=A1[:], in_=WG4[:],
                                         axis=mybir.AxisListType.X)
                    for mt in range(NT):
                        a_ps = ps_a.tile([128, O], f32, tag="a_ps")
                        nc.tensor.matmul(a_ps[:], Mblk[:], A1[:, mt, :],
                                         start=True, stop=True)
                        if first:
                            nc.vector.tensor_scalar_mul(
                                bq[:, mt, :], a_ps[:], 1.0 / B)
                        else:
                            nc.vector.scalar_tensor_tensor(
                                out=bq[:, mt, :], in0=a_ps[:], scalar=1.0 / B,
                                in1=bq[:, mt, :], op0=ALU.mult, op1=ALU.add)

    nc.compile()
    return nc


def _get_nc():
    if "nc" not in _CACHE:
        _CACHE["nc"] = _build()
    return _CACHE["nc"]


def _tile128(a):
    """[R, C] -> [128, R//128, C] with row r = t*128+p at [p, t]."""
    r, c = a.shape
    return np.ascontiguousarray(
        a.reshape(r // 128, 128, c).transpose(1, 0, 2))


def _make_in_maps(x, W):
    x = np.asarray(x, dtype=np.float32)
    W = np.asarray(W, dtype=np.float32)
    mblk = np.kron(np.eye(16, dtype=np.float32),
                   np.ones((8, 8), dtype=np.float32))
    in_maps = []
    for core in range(NCORES):
        isl = slice(core * I_SH, (core + 1) * I_SH)
        x_flat = x[:, isl, :].reshape(B, ID)
        w_flat = W[isl].transpose(0, 2, 1, 3).reshape(ID, OE)
        in_maps.append({
            "xT": _tile128(np.ascontiguousarray(x_flat.T)),
            "xf": _tile128(x_flat),
            "Wf": _tile128(w_flat),
            "Mblk": mblk,
        })
    return in_maps


def _ensure_ntff_hook():
    """This image's antenv lacks axon_hooks; reconstruct it so trace=True
    can reach the NTFF profiler in libaxon_pjrt.so."""
    import sys
    import types
    try:
        import antenv.axon_hooks  # noqa: F401
        return
    except ImportError:
        pass
    try:
        import antenv
        from trn_agent_boot.trn_boot import _ntff_profile_via_ctypes
        hook = _ntff_profile_via_ctypes("/opt/axon/libaxon_pjrt.so")
        mod = types.ModuleType("antenv.axon_hooks")
        mod._hook = hook
        mod.get_axon_ntff_profile_hook = lambda: mod._hook
        mod.set_axon_ntff_profile_hook = (
            lambda h: setattr(mod, "_hook", h))
        sys.modules["antenv.axon_hooks"] = mod
        antenv.axon_hooks = mod
    except Exception as e:  # profiling is best-effort
        print("ntff hook setup failed:", e)


def _run_hw(x, W, trace=False, **kwargs):
    from concourse import bass_utils
    if trace:
        _ensure_ntff_hook()
    nc = _get_nc()
    res = bass_utils.run_bass_kernel_spmd(
        nc, _make_in_maps(x, W), core_ids=list(range(NCORES)),
        trace=trace, **kwargs)
    out = res.results[0]["out"]
    return out.reshape(B, O, DOUT)[..., None].astype(np.float32), res


def kernel(x, W):
    out, _ = _run_hw(x, W, trace=False)
    return out


# revision 4
# speedup vs baseline: 1.2770x; 1.2770x over previous
"""CapsNet dynamic-routing kernel for 8 TRN2 NeuronCores.

Problem: x [256,1152,8], W [1152,10,8,16], 3 routing iterations, out [256,10,16,1].

Strategy (sharded over the input-capsule axis I, 144 capsules / core):
  u_hat is NEVER materialized (it would be 189MB). Instead each routing
  iteration computes, per core (ID = 144*8 = 1152 local (i,d) rows):
    s_partial[b,(o,e)] = x_flat[b,:] @ (c*W)_flat[:,(o,e)]     (dense matmul, K=ID)
    s = AllReduce(s_partial) over the 8 I-shards                (164KB, 1/iter)
    v = squash(s)            (computed redundantly on all cores)
    G[(i,d),(o,e)] = x_flat^T @ v_flat                          (dense matmul)
    agree[i,o] = (1/B) * sum_{d,e} (W_flat * G)[(i,d),(o,e)]    (local, no comm)
    b += agree ; c = softmax(b, axis=o)                         (local)
  Iteration 1 uses uniform c=1/O folded into the squash scale, so the
  c*W build is skipped there.  The last iteration skips the agreement.
  Matmul operands are bf16 (fp32 matmul runs 2 passes at 1/4 rate on
  TRN2); all accumulation stays fp32 (PSUM), the AllReduce is fp32.
  Softmax skips max-subtraction: |b| stays O(1) for this routing.
"""

import numpy as np

B, I, O, DIN, DOUT = 256, 1152, 10, 8, 16
NCORES = 8
I_SH = I // NCORES          # 144 input capsules per core
ID = I_SH * DIN             # 1152 local (i,d) rows
NT = ID // 128              # 9 partition tiles of (i,d)
BT = B // 128               # 2 partition tiles of batch
OE = O * DOUT               # 160
ROUTING_ITERS = 3

_CACHE = {}


def _bc(ap_mod, ap, n):
    """View an AP with an extra innermost broadcast axis of length n."""
    return ap_mod.AP(tensor=ap.tensor, offset=ap.offset, ap=[*ap.ap, [0, n]])


def _build():
    import concourse.bass as bass
    import concourse.bacc as bacc
    import concourse.tile as tile
    from concourse import mybir

    f32 = mybir.dt.float32
    bf16 = mybir.dt.bfloat16
    AF = mybir.ActivationFunctionType
    ALU = mybir.AluOpType

    nc = bacc.Bacc("TRN2", target_bir_lowering=False, debug=False,
                   num_devices=NCORES)

    # All inputs pre-tiled on host to [128, ...] so every DMA is contiguous.
    xT_d = nc.dram_tensor("xT", [128, NT, B], bf16, kind="ExternalInput")
    xf_d = nc.dram_tensor("xf", [128, BT, ID], bf16, kind="ExternalInput")
    W_d = nc.dram_tensor("Wf", [128, NT, OE], f32, kind="ExternalInput")
    Wb_d = nc.dram_tensor("Wb", [128, NT, OE], bf16, kind="ExternalInput")
    MB_d = nc.dram_tensor("Mblk", [128, 128], bf16, kind="ExternalInput")
    out_d = nc.dram_tensor("out", [B, OE], f32, kind="ExternalOutput")

    with tile.TileContext(nc) as tc:
        with (
            tc.tile_pool(name="sb", bufs=1) as sb,
            tc.tile_pool(name="work", bufs=2) as work,
            tc.tile_pool(name="ps_s", bufs=2, space="PSUM") as ps_s,
            tc.tile_pool(name="ps_g", bufs=2, space="PSUM") as ps_g,
            tc.tile_pool(name="ps_a", bufs=2, space="PSUM") as ps_a,
            tc.tile_pool(name="dram", bufs=3, space="DRAM") as dram,
        ):
            # ---- persistent SBUF tensors ----
            xT = sb.tile([128, NT, B], bf16)      # x_flat^T tiles (lhsT for s)
            xf = sb.tile([128, BT, ID], bf16)     # x_flat tiles (lhsT for G)
            Wf = sb.tile([128, NT, OE], f32)      # W_flat tiles (agree path)
            Wb = sb.tile([128, NT, OE], bf16)     # W_flat bf16 (iter-1 rhs)
            Mblk = sb.tile([128, 128], bf16)      # 8x8 block-diag ones
            bq = sb.tile([128, NT, O], f32)       # routing logits b (expanded)
            csb = sb.tile([128, NT, O], f32)      # softmax(b)
            Wc = sb.tile([128, NT, OE], bf16)     # c * W
            s_sb = sb.tile([128, BT, OE], f32)    # local partial s
            sf = sb.tile([128, BT, OE], f32)      # all-reduced s
            vb = sb.tile([128, BT, OE], bf16)     # squash(s) bf16 (rhs for G)
            vf = sb.tile([128, BT, OE], f32)      # squash(s) f32 (final out)
            WG = sb.tile([128, NT, OE], f32)      # W * G
            A1 = sb.tile([128, NT, O], f32)       # e-reduced agreement
            A1b = sb.tile([128, NT, O], bf16)     # bf16 copy (rhs for d-sum)

            nc.sync.dma_start(out=xT[:], in_=xT_d[:])
            nc.sync.dma_start(out=xf[:], in_=xf_d[:])
            nc.sync.dma_start(out=Wf[:], in_=W_d[:])
            nc.sync.dma_start(out=Wb[:], in_=Wb_d[:])
            nc.sync.dma_start(out=Mblk[:], in_=MB_d[:])

            Wf4 = Wf.rearrange("p t (o e) -> p t o e", o=O)
            Wc4 = Wc.rearrange("p t (o e) -> p t o e", o=O)
            WG4 = WG.rearrange("p t (o e) -> p t o e", o=O)
            sf4 = sf.rearrange("p b (o e) -> p b o e", o=O)
            vb4 = vb.rearrange("p b (o e) -> p b o e", o=O)
            vf4 = vf.rearrange("p b (o e) -> p b o e", o=O)

            for it in range(ROUTING_ITERS):
                first, last = it == 0, it == ROUTING_ITERS - 1

                if first:
                    rhs = Wb      # uniform c = 1/O folded into squash scale
                else:
                    # c = softmax(b) over o per (i,d) row; |b| is O(1) so no
                    # max-subtraction is needed (matches jax softmax exactly
                    # up to rounding).
                    ex = work.tile([128, NT, O], f32, tag="ex")
                    nc.scalar.activation(out=ex[:], in_=bq[:], func=AF.Exp)
                    sm = work.tile([128, NT], f32, tag="sm")
                    nc.vector.reduce_sum(out=sm[:], in_=ex[:],
                                         axis=mybir.AxisListType.X)
                    nc.vector.reciprocal(out=sm[:], in_=sm[:])
                    nc.vector.tensor_tensor(
                        out=csb[:], in0=ex[:], in1=_bc(bass, sm[:], O),
                        op=ALU.mult)
                    # Wc = W * c  (c broadcast over e), cast to bf16
                    nc.vector.tensor_tensor(
                        out=Wc4[:], in0=Wf4[:],
                        in1=_bc(bass, csb[:], DOUT), op=ALU.mult)
                    rhs = Wc

                # s_partial = x_flat @ rhs : out [b-tile 128, OE]
                for bt in range(BT):
                    s_ps = ps_s.tile([128, OE], f32, tag="s_ps")
                    for k in range(NT):
                        nc.tensor.matmul(
                            s_ps[:],
                            xT[:, k, bt * 128:(bt + 1) * 128],
                            rhs[:, k, :],
                            start=(k == 0), stop=(k == NT - 1))
                    nc.vector.tensor_copy(s_sb[:, bt, :], s_ps[:])

                # AllReduce s over the 8 I-shards
                cc_in = dram.tile([BT, 128, OE], f32, tag="cc_in")
                cc_out = dram.tile([BT, 128, OE], f32, tag="cc_out",
                                   addr_space="Shared")
                nc.sync.dma_start(
                    out=cc_in.rearrange("b p f -> p b f"), in_=s_sb[:])
                nc.gpsimd.collective_compute(
                    "AllReduce", ALU.add,
                    replica_groups=[list(range(NCORES))],
                    ins=[cc_in.opt()], outs=[cc_out.opt()])
                nc.sync.dma_start(
                    out=sf[:], in_=cc_out.rearrange("b p f -> p b f"))

                # squash: v = s * sqrt(ss)/(1+ss) per (b, o); iteration 1
                # carries c=1/O as s_raw = O*s_true.
                sq = work.tile([128, BT, OE], f32, tag="sq")
                nc.vector.tensor_tensor(out=sq[:], in0=sf[:], in1=sf[:],
                                        op=ALU.mult)
                ss = work.tile([128, BT, O], f32, tag="ss")
                nc.vector.reduce_sum(
                    out=ss[:], in_=sq.rearrange("p b (o e) -> p b o e", o=O),
                    axis=mybir.AxisListType.X)
                t1 = work.tile([128, BT, O], f32, tag="t1")
                nc.scalar.activation(out=t1[:], in_=ss[:], func=AF.Sqrt)
                den = work.tile([128, BT, O], f32, tag="den")
                if first:
                    # ss_raw = O^2*ss_true:
                    #   v = s_raw*(1/O^2)*sqrt(ss_raw)/(1+ss_raw/O^2)
                    nc.vector.tensor_scalar(
                        out=den[:], in0=ss[:], scalar1=1.0 / (O * O),
                        scalar2=1.0, op0=ALU.mult, op1=ALU.add)
                else:
                    nc.vector.tensor_scalar_add(den[:], ss[:], 1.0)
                nc.vector.reciprocal(out=den[:], in_=den[:])
                rat = work.tile([128, BT, O], f32, tag="rat")
                nc.vector.tensor_tensor(out=rat[:], in0=t1[:], in1=den[:],
                                        op=ALU.mult)
                if first:
                    nc.vector.tensor_scalar_mul(rat[:], rat[:], 1.0 / (O * O))
                vout4 = vf4 if last else vb4
                nc.vector.tensor_tensor(
                    out=vout4[:], in0=sf4[:],
                    in1=_bc(bass, rat[:], DOUT), op=ALU.mult)

                if last:
                    nc.sync.dma_start(
                        out=out_d.rearrange("(b p) f -> p b f", p=128),
                        in_=vf[:])
                else:
                    # G = x_flat^T @ v ; agree = (1/B) sum_de W*G ; b += agree
                    for mt in range(NT):
                        g_ps = ps_g.tile([128, OE], f32, tag="g_ps")
                        for bt in range(BT):
                            nc.tensor.matmul(
                                g_ps[:],
                                xf[:, bt, mt * 128:(mt + 1) * 128],
                                vb[:, bt, :],
                                start=(bt == 0), stop=(bt == BT - 1))
                        nc.vector.tensor_tensor(
                            out=WG[:, mt, :], in0=Wf[:, mt, :], in1=g_ps[:],
                            op=ALU.mult)
                    nc.vector.reduce_sum(out=A1[:], in_=WG4[:],
                                         axis=mybir.AxisListType.X)
                    nc.vector.tensor_copy(A1b[:], A1[:])
                    for mt in range(NT):
                        a_ps = ps_a.tile([128, O], f32, tag="a_ps")
                        nc.tensor.matmul(a_ps[:], Mblk[:], A1b[:, mt, :],
                                         start=True, stop=True)
                        if first:
                            nc.vector.tensor_scalar_mul(
                                bq[:, mt, :], a_ps[:], 1.0 / B)
                        else:
                            nc.vector.scalar_tensor_tensor(
                                out=bq[:, mt, :], in0=a_ps[:], scalar=1.0 / B,
                                in1=bq[:, mt, :], op0=ALU.mult, op1=ALU.add)

    nc.compile()
    return nc


def _get_nc():
    if "nc" not in _CACHE:
        _CACHE["nc"] = _build()
    return _CACHE["nc"]


def _tile128(a):
    """[R, C] -> [128, R//128, C] with row r = t*128+p at [p, t]."""
    r, c = a.shape
    return np.ascontiguousarray(
        a.reshape(r // 128, 128, c).transpose(1, 0, 2))


def _make_in_maps(x, W):
    from concourse import mybir
    bfdt = mybir.dt.np(mybir.dt.bfloat16)
    x = np.asarray(x, dtype=np.float32)
    W = np.asarray(W, dtype=np.float32)
    mblk = np.kron(np.eye(16, dtype=np.float32),
                   np.ones((8, 8), dtype=np.float32)).astype(bfdt)
    in_maps = []
    for core in range(NCORES):
        isl = slice(core * I_SH, (core + 1) * I_SH)
        x_flat = x[:, isl, :].reshape(B, ID)
        w_flat = W[isl].transpose(0, 2, 1, 3).reshape(ID, OE)
        wt = _tile128(w_flat)
        in_maps.append({
            "xT": _tile128(np.ascontiguousarray(x_flat.T)).astype(bfdt),
            "xf": _tile128(x_flat).astype(bfdt),
            "Wf": wt,
            "Wb": wt.astype(bfdt),
            "Mblk": mblk,
        })
    return in_maps


def _ensure_ntff_hook():
    """This image's antenv lacks axon_hooks; reconstruct it so trace=True
    can reach the NTFF profiler in libaxon_pjrt.so."""
    import sys
    import types
    try:
        import antenv.axon_hooks  # noqa: F401
        return
    except ImportError:
        pass
    try:
        import antenv
        from trn_agent_boot.trn_boot import _ntff_profile_via_ctypes
        hook = _ntff_profile_via_ctypes("/opt/axon/libaxon_pjrt.so")
        mod = types.ModuleType("antenv.axon_hooks")
        mod._hook = hook
        mod.get_axon_ntff_profile_hook = lambda: mod._hook
        mod.set_axon_ntff_profile_hook = (
            lambda h: setattr(mod, "_hook", h))
        sys.modules["antenv.axon_hooks"] = mod
        antenv.axon_hooks = mod
    except Exception as e:  # profiling is best-effort
        print("ntff hook setup failed:", e)


def _run_hw(x, W, trace=False, **kwargs):
    from concourse import bass_utils
    if trace:
        _ensure_ntff_hook()
    nc = _get_nc()
    res = bass_utils.run_bass_kernel_spmd(
        nc, _make_in_maps(x, W), core_ids=list(range(NCORES)),
        trace=trace, **kwargs)
    out = res.results[0]["out"]
    return out.reshape(B, O, DOUT)[..., None].astype(np.float32), res


def kernel(x, W):
    out, _ = _run_hw(x, W, trace=False)
    return out
